# revision 1
# baseline (speedup 1.0000x reference)
"""Trainium2 Bass kernel for nn_EncoderLayer (B=2, L=2048, D=1024, 16 heads, FFN 4096).

Strategy: sequence-parallel over the 8 cores (core c owns batch c//4, query rows
(c%4)*512 .. +512).  Each core recomputes the full K projection for its batch
(4.3 GF duplicated work) which avoids all collectives; everything else is local.

Device layout: activations are kept transposed (features on partitions, tokens on
the free dim) so every matmul contracts over the partition dim.  The reference
interprets the projection output as [l, HD, N] (feature = d*16 + h), so the host
permutes wq/wk columns and wo rows to make heads contiguous 64-blocks; outputs
come back in natural feature order.

Numerics: matmuls in bf16 with fp32 PSUM accumulation; softmax, layernorm, gelu
in fp32 on ACT/DVE.  Softmax normalization uses an appended ones-column in the
K operand so the row-sum falls out of the same matmul that computes attn @ K.

v2 optimizations (610us -> ~455us HW):
- kaug derived from kT via PE transposes instead of a second matmul pass;
  kaug padded so the ctx stationary is always 128 columns (FWL-eligible).
- Softmax denominators batched per 8-head half: one fast approximate
  reciprocal + a 2-DMA broadcast + one multiply (vs 16 serial reciprocals).
- LN1 prep (bf16 copy, square, partition-sums) folded into the wo loop; LN2
  prep folded into the FFN output loop; LN mu/rstd broadcast via a PE
  ones-column matmul into PSUM (no DRAM round-trip); chunk-pair normalizes.
- wo prefetched during attention; w2 prefetched during FFN half 0
  (interleaved with the w1 stream so neither starves the DMA queue);
  1/sqrt(HD) folded into wq on the host; wq co-blocked + xb mt-blocked DMA
  layouts so the first matmul starts ~5us in; chunked output DMA.
"""

import sys
sys.setrecursionlimit(200000)
import numpy as np
import ml_dtypes

B, L, D, NH, HD, FF = 2, 2048, 1024, 16, 64, 4096
LQ = 512  # query rows per core
NCORES = 8
EPS = 1e-5
DC = D // 128  # 8 feature chunks
MC = L // 128  # 16 key chunks
FC = FF // 128  # 32 ffn chunks
BF16NP = ml_dtypes.bfloat16

_cache = {}
LAST_RESULTS = None


def _build_nc():
    import concourse.bass as bass
    import concourse.tile as tile
    from concourse import bacc, mybir
    from contextlib import ExitStack

    f32 = mybir.dt.float32
    bf16 = mybir.dt.bfloat16
    AF = mybir.ActivationFunctionType
    OP = mybir.AluOpType

    nc = bacc.Bacc("TRN2", debug=False, target_bir_lowering=False)

    # ---- DRAM I/O ----
    xb_d = nc.dram_tensor("xb", [4, D, 512], bf16, kind="ExternalInput").ap()
    xqb_d = nc.dram_tensor("xqb", [D, LQ], bf16, kind="ExternalInput").ap()
    xq_d = nc.dram_tensor("xq", [D, LQ], f32, kind="ExternalInput").ap()
    wq_d = nc.dram_tensor("wq", [DC, 128, D], bf16, kind="ExternalInput").ap()
    wk_d = nc.dram_tensor("wk", [D, D], bf16, kind="ExternalInput").ap()
    wo_d = nc.dram_tensor("wo", [D, D], bf16, kind="ExternalInput").ap()
    w1_d = nc.dram_tensor("w1", [FC, 128, D], bf16, kind="ExternalInput").ap()
    w2_d = nc.dram_tensor("w2", [FC, 128, D], bf16, kind="ExternalInput").ap()
    ident_d = nc.dram_tensor("ident", [128, 128], bf16, kind="ExternalInput").ap()
    bb1_d = nc.dram_tensor("bb1", [FF], f32, kind="ExternalInput").ap()
    bb2_d = nc.dram_tensor("bb2", [D], f32, kind="ExternalInput").ap()
    g1_d = nc.dram_tensor("g1", [D], f32, kind="ExternalInput").ap()
    b1_d = nc.dram_tensor("b1", [D], f32, kind="ExternalInput").ap()
    g2_d = nc.dram_tensor("g2", [D], f32, kind="ExternalInput").ap()
    b2_d = nc.dram_tensor("b2", [D], f32, kind="ExternalInput").ap()
    out_d = nc.dram_tensor("out", [D, LQ], f32, kind="ExternalOutput").ap()

    xqb_v = xqb_d.rearrange("(c p) l -> p c l", p=128)
    xq_v = xq_d.rearrange("(c p) l -> p c l", p=128)
    wk_v = wk_d.rearrange("(c p) f -> p c f", p=128)
    wo_v = wo_d.rearrange("(c p) f -> p c f", p=128)
    bb1_v = bb1_d.rearrange("(c p) -> p c", p=128)
    bb2_v = bb2_d.rearrange("(c p) -> p c", p=128)
    g1_v = g1_d.rearrange("(c p) -> p c", p=128)
    b1_v = b1_d.rearrange("(c p) -> p c", p=128)
    g2_v = g2_d.rearrange("(c p) -> p c", p=128)
    b2_v = b2_d.rearrange("(c p) -> p c", p=128)
    out_v = out_d.rearrange("(c p) l -> p c l", p=128)

    with tile.TileContext(nc, pool_alloc_mode="queue") as tc, ExitStack() as top:
        consts = top.enter_context(tc.tile_pool(name="consts", bufs=1))
        dramsc = top.enter_context(tc.tile_pool(name="dramsc", bufs=2, space="DRAM"))

        sm = top.enter_context(tc.tile_pool(name="smalls", bufs=1))
        sm2 = top.enter_context(tc.tile_pool(name="smalls2", bufs=2))

        with tc.tile_pool(name="mid", bufs=1) as mid:
            hT = mid.tile([128, DC, LQ], f32, tag="hT")
            hb = mid.tile([128, DC, LQ], bf16, tag="hb")

            with tc.tile_pool(name="kq", bufs=1) as kq:
                kT = kq.tile([128, DC, L], bf16, tag="kT")
                kaug = kq.tile([128, MC, NH * 65 + 63], bf16, tag="kaug")
                qT = kq.tile([128, DC, LQ], bf16, tag="qT")
                kaug_h = kaug[:, :, 0:NH * 65].rearrange("p m (h e) -> p m h e", e=65)

                # ---- Phase 1: projections ----
                with tc.tile_pool(name="p1", bufs=1) as p1, \
                     tc.tile_pool(name="p1w", bufs=1) as p1w, \
                     tc.tile_pool(name="psA", bufs=4, space="PSUM") as psA:
                    # q-path inputs first so the tensor engine starts early
                    xqb = p1.tile([128, DC, LQ], bf16, tag="xqb")
                    nc.sync.dma_start(xqb, xqb_v)
                    wq_sb = p1w.tile([128, DC, D], bf16, tag="wproj")
                    for co in range(DC):
                        nc.sync.dma_start(wq_sb[:, co, :], wq_d[co])
                    wk_sb = p1w.tile([128, DC, D], bf16, tag="wproj_k")
                    nc.sync.dma_start(wk_sb, wk_v)
                    xb = p1.tile([128, DC, L], bf16, tag="xb")
                    for mt in range(4):
                        nc.sync.dma_start(
                            xb[:, :, mt * 512:(mt + 1) * 512],
                            xb_d[mt].rearrange("(c p) m -> p c m", p=128))
                    ident = consts.tile([128, 128], bf16, tag="ident")
                    nc.sync.dma_start(ident, ident_d)

                    # constants (small DMAs, off the critical path)
                    ones_bf = consts.tile([128, 1], bf16, tag="ones")
                    nc.vector.memset(ones_bf, 1.0)
                    ones_row = consts.tile([1, 128], f32, tag="ones_row")
                    nc.vector.memset(ones_row, 1.0)
                    eps_t = consts.tile([1, 1], f32, tag="eps")
                    nc.vector.memset(eps_t, EPS)
                    bb1_sb = consts.tile([128, FC], f32, tag="bb1")
                    nc.sync.dma_start(bb1_sb, bb1_v)
                    bb2_sb = consts.tile([128, DC], f32, tag="bb2")
                    nc.sync.dma_start(bb2_sb, bb2_v)
                    g1_sb = consts.tile([128, DC], f32, tag="g1")
                    nc.sync.dma_start(g1_sb, g1_v)
                    b1_sb = consts.tile([128, DC], f32, tag="b1")
                    nc.sync.dma_start(b1_sb, b1_v)
                    g2_sb = consts.tile([128, DC], f32, tag="g2")
                    nc.sync.dma_start(g2_sb, g2_v)
                    b2_sb = consts.tile([128, DC], f32, tag="b2")
                    nc.sync.dma_start(b2_sb, b2_v)
                    nc.vector.memset(kaug_h[:, :, :, 64:65], 1.0)
                    nc.vector.memset(kaug[:, :, NH * 65:], 0.0)

                    # qT = (x_q @ wq)^T  (1/sqrt(HD) folded into wq on the host)
                    for co in range(DC):
                        ps = psA.tile([128, 512], f32, tag="ps")
                        for c in range(DC):
                            nc.tensor.matmul(ps, wq_sb[:, co, c * 128:(c + 1) * 128],
                                             xqb[:, c, :], start=(c == 0), stop=(c == DC - 1))
                        nc.vector.tensor_copy(qT[:, co, :], ps)

                    # kT = (x @ wk)^T  over the full sequence
                    for mt in range(L // 512):
                        for co in range(DC):
                            ps = psA.tile([128, 512], f32, tag="ps")
                            for c in range(DC):
                                nc.tensor.matmul(ps, wk_sb[:, c, co * 128:(co + 1) * 128],
                                                 xb[:, c, mt * 512:(mt + 1) * 512],
                                                 start=(c == 0), stop=(c == DC - 1))
                            nc.vector.tensor_copy(kT[:, co, mt * 512:(mt + 1) * 512], ps)

                    # kaug = kT^T via PE transposes (4 feature-chunks per PSUM tile)
                    with tc.tile_pool(name="psT", bufs=4, space="PSUM") as psT:
                        for mi in range(MC):
                            for g in range(2):
                                pt = psT.tile([128, 512], bf16, tag="pt")
                                for j in range(4):
                                    c = g * 4 + j
                                    nc.tensor.transpose(
                                        pt[:, j * 128:(j + 1) * 128],
                                        kT[:, c, mi * 128:(mi + 1) * 128], ident)
                                nc.vector.tensor_copy(
                                    kaug_h[:, mi, g * 8:(g + 1) * 8, 0:64],
                                    pt.rearrange("p (h e) -> p h e", e=64))

                # ---- Phase 2: attention ----
                with tc.tile_pool(name="r1p", bufs=1) as r1p, \
                     tc.tile_pool(name="psL1", bufs=1, space="PSUM") as psL1:
                  s1_ps = psL1.tile([1, LQ], f32, tag="ln1_sum_r")
                  q1_ps = psL1.tile([1, LQ], f32, tag="ln1_sum_s")
                  with tc.tile_pool(name="attn", bufs=1) as attn, \
                       tc.tile_pool(name="epool", bufs=3) as epool, \
                       tc.tile_pool(name="wop", bufs=1) as wop:
                    with tc.tile_pool(name="psS", bufs=2, space="PSUM") as psS, \
                         tc.tile_pool(name="psU", bufs=2, space="PSUM") as psU:
                      r1T = r1p.tile([128, DC, LQ], f32, tag="r1T")
                      # prefetch wo while attention runs
                      wo_sb = wop.tile([128, DC, D], bf16, tag="wo_sb")
                      nc.sync.dma_start(wo_sb, wo_v)

                      ctxT = attn.tile([128, DC, LQ], bf16, tag="ctxT")
                      dens2 = [attn.tile([8, LQ], f32, tag="densA", name="densA"),
                               attn.tile([8, LQ], f32, tag="densB", name="densB")]

                      for h in range(NH):
                          pair, poff = h // 2, 64 * (h % 2)
                          e_tiles = []
                          for half in range(2):
                              e = epool.tile([128, 8, LQ], bf16, tag="E")
                              for mt in range(4):
                                  st = psS.tile([128, 1024], f32, tag="st")
                                  for j in range(2):
                                      mi = half * 8 + mt * 2 + j
                                      nc.tensor.matmul(
                                          st[:, j * 512:(j + 1) * 512],
                                          kT[poff:poff + 64, pair, mi * 128:(mi + 1) * 128],
                                          qT[poff:poff + 64, pair, :],
                                          start=True, stop=True)
                                  nc.scalar.activation(
                                      e[:, mt * 2:(mt + 1) * 2, :].rearrange("p a b -> p (a b)"),
                                      st, AF.Exp)
                              e_tiles.append(e)
                          u = psU.tile([128, 512], f32, tag="u")
                          for mi in range(MC):
                              nc.tensor.matmul(u, kaug[:, mi, h * 65:h * 65 + 128],
                                               e_tiles[mi // 8][:, mi % 8, :],
                                               start=(mi == 0), stop=(mi == MC - 1))
                          nc.vector.tensor_copy(ctxT[poff:poff + 64, pair, :],
                                                u[0:64, :])
                          drow = sm2.tile([1, LQ], f32, tag="drow")
                          nc.vector.tensor_copy(drow, u[64:65, :])
                          nc.sync.dma_start(dens2[h // 8][h % 8:h % 8 + 1, :], drow)
                          if h % 8 == 7:
                              # half-batch denominators: reciprocal + bcast + multiply
                              hb0 = h - 7
                              rec32 = attn.tile([8, LQ], f32, tag="rec32")
                              nc.vector.reciprocal_approx_fast(
                                  rec32, dens2[h // 8])
                              rec16 = attn.tile([8, LQ], bf16, tag="rec16")
                              nc.vector.tensor_copy(rec16, rec32)
                              scd = dramsc.tile([8, LQ], bf16, tag="rec_sc",
                                                name=f"rec_sc{hb0}")
                              nc.sync.dma_start(scd, rec16)
                              scd_v = scd.rearrange("(c two) l -> two c l", two=2)
                              cl = slice(hb0 // 2, hb0 // 2 + 4)
                              den_bc = attn.tile([128, 4, LQ], bf16, tag="den_bc")
                              nc.sync.dma_start(den_bc[0:64],
                                                scd_v[0].partition_broadcast(64))
                              nc.sync.dma_start(den_bc[64:128],
                                                scd_v[1].partition_broadcast(64))
                              nc.vector.tensor_tensor(
                                  ctxT[:, cl, :].rearrange("p c l -> p (c l)"),
                                  ctxT[:, cl, :].rearrange("p c l -> p (c l)"),
                                  den_bc.rearrange("p c l -> p (c l)"), OP.mult)


                    # attn_out + residual -> r1T, with LN1 prep folded in
                    with tc.tile_pool(name="psB", bufs=4, space="PSUM") as psB:
                          for f in range(DC):
                              xq_t = sm2.tile([128, 512], f32, tag="xq_t")
                              nc.sync.dma_start(xq_t, xq_v[:, f, :])
                              ps = psB.tile([128, 512], f32, tag="ao")
                              for c in range(DC):
                                  nc.tensor.matmul(ps, wo_sb[:, c, f * 128:(f + 1) * 128],
                                                   ctxT[:, c, :], start=(c == 0), stop=(c == DC - 1))
                              nc.vector.tensor_tensor(r1T[:, f, :], ps, xq_t, OP.add)
                              rb1 = sm2.tile([128, 512], bf16, tag="rb1")
                              nc.vector.tensor_copy(rb1, r1T[:, f, :])
                              sq1 = sm2.tile([128, 512], bf16, tag="sq1")
                              nc.vector.tensor_tensor(sq1, rb1, rb1, OP.mult)
                              nc.tensor.matmul(s1_ps, ones_bf, rb1,
                                               start=(f == 0), stop=(f == DC - 1))
                              nc.tensor.matmul(q1_ps, ones_bf, sq1,
                                               start=(f == 0), stop=(f == DC - 1))

                  # ---- LN1 stats + normalize (chunk pairs) -> hT, hb ----
                  with tc.tile_pool(name="cen1p", bufs=2) as cen1p, \
                       tc.tile_pool(name="psM1", bufs=1, space="PSUM") as psM1:
                      mu = sm.tile([1, LQ], f32, tag="ln_mu")
                      nc.scalar.activation(mu, s1_ps, AF.Copy, scale=1.0 / D)
                      msq = sm.tile([1, LQ], f32, tag="ln_msq")
                      nc.scalar.activation(msq, q1_ps, AF.Copy, scale=1.0 / D)
                      var = sm.tile([1, LQ], f32, tag="ln_var")
                      nc.vector.tensor_tensor(var, mu, mu, OP.mult)
                      nc.vector.tensor_tensor(var, msq, var, OP.subtract)
                      std = sm.tile([1, LQ], f32, tag="ln_std")
                      nc.scalar.activation(std, var, AF.Sqrt, bias=eps_t)
                      mrrow = sm.tile([1, 2 * LQ], f32, tag="ln_mrrow")
                      nc.vector.reciprocal_approx_fast(mrrow[:, LQ:2 * LQ], std)
                      nc.vector.tensor_copy(mrrow[:, 0:LQ], mu)
                      mr_ps = psM1.tile([128, 2 * LQ], f32, tag="ln_mrps")
                      for j in range(2):
                          nc.tensor.matmul(mr_ps[:, j * LQ:(j + 1) * LQ], ones_row,
                                           mrrow[:, j * LQ:(j + 1) * LQ],
                                           start=True, stop=True)
                      mu_bc, rstd_bc = mr_ps[:, 0:LQ], mr_ps[:, LQ:2 * LQ]
                      for c in range(DC):
                          cen = cen1p.tile([128, LQ], f32, tag="ln_cen")
                          nc.vector.tensor_tensor(cen, r1T[:, c, :], mu_bc, OP.subtract)
                          nc.vector.tensor_tensor(cen, cen, rstd_bc, OP.mult)
                          nc.scalar.activation(hT[:, c, :], cen, AF.Identity,
                                               scale=g1_sb[:, c:c + 1], bias=b1_sb[:, c:c + 1])
                          nc.vector.tensor_copy(hb[:, c, :], hT[:, c, :])
            # ---- Phase 3: FFN ----
            with tc.tile_pool(name="ffn", bufs=1) as ffn, \
                 tc.tile_pool(name="w1stream", bufs=4) as w1stream, \
                 tc.tile_pool(name="w2pool", bufs=1) as w2pool:
                g_sb = ffn.tile([128, FC, LQ], bf16, tag="g")
                r2T = ffn.tile([128, DC, LQ], f32, tag="r2T")
                w2_sb = w2pool.tile([128, FC, D], bf16, tag="w2_sb")

                with tc.tile_pool(name="psL2", bufs=1, space="PSUM") as psL2:
                  s2_ps = psL2.tile([1, LQ], f32, tag="ln2_sum_r")
                  q2_ps = psL2.tile([1, LQ], f32, tag="ln2_sum_s")
                  with tc.tile_pool(name="psZO", bufs=1, space="PSUM") as psZO:
                    for half in range(2):
                        o_ps = [psZO.tile([128, 512], f32, tag=f"o{f}", name=f"o_ps{f}")
                                for f in range(4)]
                        for i in range(FC):
                            if half == 0:
                                w1t = w1stream.tile([128, D], bf16, tag="w1t")
                                nc.sync.dma_start(w1t, w1_d[i])
                                nc.sync.dma_start(w2_sb[:, i, :], w2_d[i])
                                zt = psZO.tile([128, 512], f32, tag=f"zt{i % 2}",
                                               name=f"zt{i % 2}")
                                for c in range(DC):
                                    nc.tensor.matmul(zt, w1t[:, c * 128:(c + 1) * 128],
                                                     hb[:, c, :], start=(c == 0), stop=(c == DC - 1))
                                nc.scalar.activation(g_sb[:, i, :], zt, AF.Gelu,
                                                     bias=bb1_sb[:, i:i + 1])
                            for f in range(4):
                                nc.tensor.matmul(o_ps[f],
                                                 w2_sb[:, i, half * 512 + f * 128:half * 512 + (f + 1) * 128],
                                                 g_sb[:, i, :], start=(i == 0), stop=(i == FC - 1))
                        for f in range(4):
                            fo = half * 4 + f
                            t = sm2.tile([128, 512], f32, tag="obias")
                            nc.scalar.activation(t, o_ps[f], AF.Identity, bias=bb2_sb[:, fo:fo + 1])
                            nc.vector.tensor_tensor(r2T[:, fo, :], t, hT[:, fo, :], OP.add)
                            # LN2 prep folded in: bf16 copy + square + partial sums
                            rb2 = sm2.tile([128, 512], bf16, tag="rb2")
                            nc.vector.tensor_copy(rb2, r2T[:, fo, :])
                            sq2 = sm2.tile([128, 512], bf16, tag="sq2")
                            nc.vector.tensor_tensor(sq2, rb2, rb2, OP.mult)
                            nc.tensor.matmul(s2_ps, ones_bf, rb2,
                                             start=(fo == 0), stop=(fo == D // 128 - 1))
                            nc.tensor.matmul(q2_ps, ones_bf, sq2,
                                             start=(fo == 0), stop=(fo == D // 128 - 1))

                  # ---- LN2 stats + normalize -> out (chunked DMA) ----
                  with tc.tile_pool(name="ln2out", bufs=3) as ln2out, \
                       tc.tile_pool(name="psM2", bufs=1, space="PSUM") as psM2:
                      mu = sm.tile([1, LQ], f32, tag="ln_mu")
                      nc.scalar.activation(mu, s2_ps, AF.Copy, scale=1.0 / D)
                      msq = sm.tile([1, LQ], f32, tag="ln_msq")
                      nc.scalar.activation(msq, q2_ps, AF.Copy, scale=1.0 / D)
                      var = sm.tile([1, LQ], f32, tag="ln_var")
                      nc.vector.tensor_tensor(var, mu, mu, OP.mult)
                      nc.vector.tensor_tensor(var, msq, var, OP.subtract)
                      std = sm.tile([1, LQ], f32, tag="ln_std")
                      nc.scalar.activation(std, var, AF.Sqrt, bias=eps_t)
                      mrrow = sm.tile([1, 2 * LQ], f32, tag="ln_mrrow")
                      nc.vector.reciprocal_approx_fast(mrrow[:, LQ:2 * LQ], std)
                      nc.vector.tensor_copy(mrrow[:, 0:LQ], mu)
                      mr_ps = psM2.tile([128, 2 * LQ], f32, tag="ln_mrps")
                      for j in range(2):
                          nc.tensor.matmul(mr_ps[:, j * LQ:(j + 1) * LQ], ones_row,
                                           mrrow[:, j * LQ:(j + 1) * LQ],
                                           start=True, stop=True)
                      mu_bc, rstd_bc = mr_ps[:, 0:LQ], mr_ps[:, LQ:2 * LQ]
                      for c in range(DC):
                          cen = ln2out.tile([128, LQ], f32, tag="ln_cen")
                          nc.vector.tensor_tensor(cen, r2T[:, c, :], mu_bc, OP.subtract)
                          nc.vector.tensor_tensor(cen, cen, rstd_bc, OP.mult)
                          oc = ln2out.tile([128, LQ], f32, tag="ln_oc")
                          nc.scalar.activation(oc, cen, AF.Identity,
                                               scale=g2_sb[:, c:c + 1], bias=b2_sb[:, c:c + 1])
                          nc.sync.dma_start(out_v[:, c, :], oc)

    nc.compile()
    return nc


def _get_nc():
    if "nc" not in _cache:
        _cache["nc"] = _build_nc()
    return _cache["nc"]


def _host_prep(inputs):
    x = np.asarray(inputs["x"], np.float32)
    wq = np.asarray(inputs["wq"], np.float32)
    wk = np.asarray(inputs["wk"], np.float32)
    wo = np.asarray(inputs["wo"], np.float32)
    g1 = np.asarray(inputs["g1"], np.float32)
    b1 = np.asarray(inputs["b1"], np.float32)
    w1 = np.asarray(inputs["w1"], np.float32)
    bb1 = np.asarray(inputs["bb1"], np.float32)
    w2 = np.asarray(inputs["w2"], np.float32)
    bb2 = np.asarray(inputs["bb2"], np.float32)
    g2 = np.asarray(inputs["g2"], np.float32)
    b2 = np.asarray(inputs["b2"], np.float32)

    idx = np.arange(D)
    perm = (idx % HD) * NH + (idx // HD)  # f' = h*64+d  ->  old f = d*16+h

    def bf(a):
        return np.ascontiguousarray(a).astype(BF16NP)

    w1t = w1.reshape(DC, 128, FC, 128).transpose(2, 1, 0, 3).reshape(FC, 128, D)
    w2t = w2.reshape(FC, 128, D)
    shared = {
        "wq": bf((wq[:, perm] * (1.0 / np.sqrt(HD))).reshape(DC, 128, DC, 128)
                 .transpose(2, 1, 0, 3).reshape(DC, 128, D)),
        "wk": bf(wk[:, perm]),
        "wo": bf(wo[perm, :]),
        "w1": bf(w1t), "w2": bf(w2t),
        "ident": bf(np.eye(128, dtype=np.float32)),
        "bb1": bb1, "bb2": bb2, "g1": g1, "b1": b1, "g2": g2, "b2": b2,
    }
    in_maps = []
    for c in range(NCORES):
        b, q0 = c // (NCORES // B), (c % (NCORES // B)) * LQ
        xT = np.ascontiguousarray(x[b].T)
        m = dict(shared)
        m["xb"] = bf(np.ascontiguousarray(
            xT.reshape(D, 4, 512).transpose(1, 0, 2)))
        m["xqb"] = bf(xT[:, q0:q0 + LQ])
        m["xq"] = np.ascontiguousarray(xT[:, q0:q0 + LQ])
        in_maps.append(m)
    return in_maps


def kernel(**inputs):
    global LAST_RESULTS
    from concourse.bass_utils import run_bass_kernel_spmd

    nc = _get_nc()
    in_maps = _host_prep(inputs)
    res = run_bass_kernel_spmd(nc, in_maps, core_ids=list(range(NCORES)))
    LAST_RESULTS = res
    out = np.empty((B, L, D), np.float32)
    for c in range(NCORES):
        b, q0 = c // (NCORES // B), (c % (NCORES // B)) * LQ
        out[b, q0:q0 + LQ, :] = res.results[c]["out"].T
    return out



# revision 11
# speedup vs baseline: 1.1571x; 1.1571x over previous
"""Trainium2 Bass kernel for nn_EncoderLayer (B=2, L=2048, D=1024, 16 heads, FFN 4096).

Strategy: sequence-parallel over the 8 cores (core c owns batch c//4, query rows
(c%4)*512 .. +512).  Each core recomputes the full K projection for its batch,
which avoids all collectives; everything else is local.

v3: fp8 (e4m3) DoubleRow matmuls for the q/k projections, attn@K, wo and w1 —
2x PE throughput (256-wide contraction per 512-cycle instruction).  Scores stay
bf16 (column-bound, no fp8 gain) and w2 stays bf16 (fp8 would eat the error
budget).  Residual/LN paths stay fp32.  The kernel is restructured as a
per-head-pair pipeline: K-proj chunk co -> PE transposes -> scores/exp/attn@K
for heads 2co,2co+1, so the ACT engine (exp is the attention bottleneck) fills
from ~6us in and PE projection work hides in ACT-bound slack.  The softmax
1/sqrt(HD) scale is applied in the exp activation (scale=0.125) instead of
pre-scaling wq, which would push fp8 weights into subnormals.
Softmax denominators: per-pair reciprocal + DRAM partition-broadcast, hidden
under the next pair's compute.  LN1 normalize is emitted interleaved with the
FFN w1 stream; LN2 tail is pipelined into chunked output DMA.
"""

import sys
sys.setrecursionlimit(200000)
import numpy as np
import ml_dtypes

B, L, D, NH, HD, FF = 2, 2048, 1024, 16, 64, 4096
LQ = 512  # query rows per core
NCORES = 8
EPS = 1e-5
DC = D // 128  # 8 feature chunks
MC = L // 128  # 16 key chunks
FC = FF // 128  # 32 ffn chunks
BF16NP = ml_dtypes.bfloat16
F8NP = ml_dtypes.float8_e4m3

_cache = {}
LAST_RESULTS = None


def _build_nc():
    import concourse.bass as bass
    import concourse.tile as tile
    from concourse import bacc, mybir
    from contextlib import ExitStack

    f32 = mybir.dt.float32
    bf16 = mybir.dt.bfloat16
    f8 = mybir.dt.float8e4
    AF = mybir.ActivationFunctionType
    OP = mybir.AluOpType
    DR = mybir.MatmulPerfMode.DoubleRow

    nc = bacc.Bacc("TRN2", debug=False, target_bir_lowering=False)

    # ---- DRAM I/O ----
    xb_d = nc.dram_tensor("xb", [4, D, 512], f8, kind="ExternalInput").ap()
    xqb_d = nc.dram_tensor("xqb", [D, LQ], f8, kind="ExternalInput").ap()
    xq_d = nc.dram_tensor("xq", [D, LQ], f32, kind="ExternalInput").ap()
    wq_d = nc.dram_tensor("wq", [DC, 128, D], f8, kind="ExternalInput").ap()
    wk_d = nc.dram_tensor("wk", [DC, 128, D], f8, kind="ExternalInput").ap()
    wo_d = nc.dram_tensor("wo", [DC, 128, D], f8, kind="ExternalInput").ap()
    w1_d = nc.dram_tensor("w1", [FC, 128, D], f8, kind="ExternalInput").ap()
    w2_d = nc.dram_tensor("w2", [FC, 128, D], bf16, kind="ExternalInput").ap()
    ident_d = nc.dram_tensor("ident", [128, 128], bf16, kind="ExternalInput").ap()
    bb1_d = nc.dram_tensor("bb1", [FF], f32, kind="ExternalInput").ap()
    bb2_d = nc.dram_tensor("bb2", [D], f32, kind="ExternalInput").ap()
    g1_d = nc.dram_tensor("g1", [D], f32, kind="ExternalInput").ap()
    b1_d = nc.dram_tensor("b1", [D], f32, kind="ExternalInput").ap()
    g2_d = nc.dram_tensor("g2", [D], f32, kind="ExternalInput").ap()
    b2_d = nc.dram_tensor("b2", [D], f32, kind="ExternalInput").ap()
    out_d = nc.dram_tensor("out", [D, LQ], f32, kind="ExternalOutput").ap()

    xqb_v = xqb_d.rearrange("(c p) l -> p c l", p=128)
    xq_v = xq_d.rearrange("(c p) l -> p c l", p=128)
    bb1_v = bb1_d.rearrange("(c p) -> p c", p=128)
    bb2_v = bb2_d.rearrange("(c p) -> p c", p=128)
    g1_v = g1_d.rearrange("(c p) -> p c", p=128)
    b1_v = b1_d.rearrange("(c p) -> p c", p=128)
    g2_v = g2_d.rearrange("(c p) -> p c", p=128)
    b2_v = b2_d.rearrange("(c p) -> p c", p=128)
    out_v = out_d.rearrange("(c p) l -> p c l", p=128)

    with tile.TileContext(nc, pool_alloc_mode="queue") as tc, ExitStack() as top:
        consts = top.enter_context(tc.tile_pool(name="consts", bufs=1))
        dramsc = top.enter_context(tc.tile_pool(name="dramsc", bufs=2, space="DRAM"))

        sm = top.enter_context(tc.tile_pool(name="smalls", bufs=1))
        sm2 = top.enter_context(tc.tile_pool(name="smalls2", bufs=2))

        with tc.tile_pool(name="mid", bufs=1) as mid:
            hT = mid.tile([128, DC, LQ], f32, tag="hT")
            hb = mid.tile([128, DC, LQ], f8, tag="hb")

            with tc.tile_pool(name="kq", bufs=1) as kq:
                kT = kq.tile([128, DC, L], bf16, tag="kT")
                # kaug[p, mj, h, i*128 + j]: m-chunk pair mj, head h, k-tile i
                # (m = 2*mj+i), col j in [0:64] = head dims, 64 = ones, rest pad
                kaug = kq.tile([128, MC // 2, NH, 256], f8, tag="kaug")
                qT = kq.tile([128, DC, LQ], bf16, tag="qT")
                ctxT = kq.tile([128, DC, LQ], f8, tag="ctxT")

                # ---- Phase 1+2: interleaved projections + attention ----
                with tc.tile_pool(name="p1", bufs=1) as p1, \
                     tc.tile_pool(name="p1w", bufs=1) as p1w, \
                     tc.tile_pool(name="epool", bufs=2) as epool, \
                     tc.tile_pool(name="cpool", bufs=2) as cpool, \
                     tc.tile_pool(name="wop", bufs=1) as wop, \
                     tc.tile_pool(name="psK", bufs=2, space="PSUM") as psK, \
                     tc.tile_pool(name="psT", bufs=1, space="PSUM") as psT, \
                     tc.tile_pool(name="psS", bufs=2, space="PSUM") as psS, \
                     tc.tile_pool(name="psU", bufs=1, space="PSUM") as psU:
                    # q-path inputs first so the tensor engine starts early
                    xqb = p1.tile([128, DC, LQ], f8, tag="xqb")
                    nc.sync.dma_start(xqb, xqb_v)
                    wq_sb = p1w.tile([128, DC, D], f8, tag="wproj")
                    for co in range(DC):
                        nc.sync.dma_start(wq_sb[:, co, :], wq_d[co])
                    wk_sb = p1w.tile([128, DC, D], f8, tag="wproj_k")
                    for co in range(DC):
                        nc.sync.dma_start(wk_sb[:, co, :], wk_d[co])
                    xb = p1.tile([128, 4, DC, 512], f8, tag="xb")
                    for mt in range(4):
                        nc.sync.dma_start(
                            xb[:, mt, :, :],
                            xb_d[mt].rearrange("(c p) m -> p c m", p=128))
                    ident = consts.tile([128, 128], bf16, tag="ident")
                    nc.sync.dma_start(ident, ident_d)

                    # constants (small DMAs, off the critical path)
                    ones_bf = consts.tile([128, 1], bf16, tag="ones")
                    nc.vector.memset(ones_bf, 1.0)
                    ones_row = consts.tile([1, 128], f32, tag="ones_row")
                    nc.vector.memset(ones_row, 1.0)
                    eps_t = consts.tile([1, 1], f32, tag="eps")
                    nc.vector.memset(eps_t, EPS)
                    bb1_sb = consts.tile([128, FC], f32, tag="bb1")
                    nc.sync.dma_start(bb1_sb, bb1_v)
                    bb2_sb = consts.tile([128, DC], f32, tag="bb2")
                    nc.sync.dma_start(bb2_sb, bb2_v)
                    g1_sb = consts.tile([128, DC], f32, tag="g1")
                    nc.sync.dma_start(g1_sb, g1_v)
                    b1_sb = consts.tile([128, DC], f32, tag="b1")
                    nc.sync.dma_start(b1_sb, b1_v)
                    g2_sb = consts.tile([128, DC], f32, tag="g2")
                    nc.sync.dma_start(g2_sb, g2_v)
                    b2_sb = consts.tile([128, DC], f32, tag="b2")
                    nc.sync.dma_start(b2_sb, b2_v)
                    kaug_b = kaug.rearrange("p mj h (two f) -> p mj (h two) f",
                                            two=2)
                    nc.vector.memset(kaug_b[:, :, :, 64:65], 1.0)
                    nc.vector.memset(kaug_b[:, :, :, 65:128], 0.0)

                    # prefetch wo during attention
                    wo_sb = wop.tile([128, DC, D], f8, tag="wo_sb")
                    for co in range(DC):
                        nc.sync.dma_start(wo_sb[:, co, :], wo_d[co])

                    scd = dramsc.tile([NH, LQ], bf16, tag="rec_sc")

                    for co in range(DC):
                        # ---- q chunk co ----
                        psq = psK.tile([128, 512], f32, tag="psk")
                        for cp in range(DC // 2):
                            nc.tensor.matmul(
                                psq,
                                wq_sb[:, co, cp * 256:(cp + 1) * 256]
                                .rearrange("p (two f) -> p two f", two=2),
                                xqb[:, 2 * cp:2 * cp + 2, :],
                                start=(cp == 0), stop=(cp == DC // 2 - 1),
                                perf_mode=DR)
                        nc.vector.tensor_copy(qT[:, co, :], psq)

                        # ---- k chunk co over full L ----
                        for mt in range(4):
                            ps = psK.tile([128, 512], f32, tag="psk")
                            for cp in range(DC // 2):
                                nc.tensor.matmul(
                                    ps,
                                    wk_sb[:, co, cp * 256:(cp + 1) * 256]
                                    .rearrange("p (two f) -> p two f", two=2),
                                    xb[:, mt, 2 * cp:2 * cp + 2, :],
                                    start=(cp == 0), stop=(cp == DC // 2 - 1),
                                    perf_mode=DR)
                            nc.vector.tensor_copy(
                                kT[:, co, mt * 512:(mt + 1) * 512], ps)

                        # ---- transposes -> kaug for heads 2co, 2co+1 ----
                        for g in range(2):
                            pt = psT.tile([128, 1024], bf16, tag="pt")
                            for j in range(8):
                                mi = g * 8 + j
                                nc.tensor.transpose(
                                    pt[:, j * 128:(j + 1) * 128],
                                    kT[:, co, mi * 128:(mi + 1) * 128], ident)
                            ptv = pt.rearrange("p (m he) -> p m he", he=128)
                            for s in range(2):
                                for i in range(2):
                                    # m-chunks g*8+i, g*8+i+2, ... (parity i)
                                    nc.vector.tensor_copy(
                                        kaug[:, g * 4:(g + 1) * 4, 2 * co + s,
                                             i * 128:i * 128 + 64],
                                        ptv[:, i::2, s * 64:(s + 1) * 64])

                        # ---- heads 2co, 2co+1 ----
                        cT = cpool.tile([128, LQ], bf16, tag="cT")
                        den_bc = cpool.tile([128, LQ], bf16, tag="den_bc")
                        for s in range(2):
                            h = 2 * co + s
                            poff = 64 * s
                            e = epool.tile([128, MC, LQ], f8, tag="E")
                            for mt in range(MC // 2):
                                st = psS.tile([128, 1024], f32, tag="st")
                                for j in range(2):
                                    mi = mt * 2 + j
                                    nc.tensor.matmul(
                                        st[:, j * 512:(j + 1) * 512],
                                        kT[poff:poff + 64, co,
                                           mi * 128:(mi + 1) * 128],
                                        qT[poff:poff + 64, co, :],
                                        start=True, stop=True)
                                nc.scalar.activation(
                                    e[:, mt * 2:(mt + 1) * 2, :]
                                    .rearrange("p a b -> p (a b)"),
                                    st, AF.Exp, scale=0.125)
                            u = psU.tile([128, 512], f32, tag="u")
                            for mj in range(MC // 2):
                                nc.tensor.matmul(
                                    u, kaug[:, mj, h, :]
                                    .rearrange("p (two f) -> p two f", two=2),
                                    e[:, 2 * mj:2 * mj + 2, :],
                                    start=(mj == 0), stop=(mj == MC // 2 - 1),
                                    perf_mode=DR)
                            nc.vector.tensor_copy(cT[poff:poff + 64, :],
                                                  u[0:64, :])
                            drow = sm2.tile([1, LQ], f32, tag="drow")
                            nc.vector.tensor_copy(drow, u[64:65, :])
                            rec32 = sm2.tile([1, LQ], f32, tag="rec32")
                            nc.vector.reciprocal_approx_fast(rec32, drow)
                            rec16 = sm2.tile([1, LQ], bf16, tag="rec16")
                            nc.vector.tensor_copy(rec16, rec32)
                            nc.sync.dma_start(scd[h:h + 1, :], rec16)
                            nc.sync.dma_start(
                                den_bc[poff:poff + 64, :],
                                scd[h:h + 1, :].partition_broadcast(64))
                        nc.vector.tensor_tensor(ctxT[:, co, :], cT, den_bc,
                                                OP.mult)

                # ---- attn_out + residual -> r1T, with LN1 prep folded in ----
                with tc.tile_pool(name="r1p", bufs=1) as r1p, \
                     tc.tile_pool(name="psL1", bufs=1, space="PSUM") as psL1, \
                     tc.tile_pool(name="cen1p", bufs=2) as cen1p, \
                     tc.tile_pool(name="psM1", bufs=1, space="PSUM") as psM1:
                    s1_ps = psL1.tile([1, LQ], f32, tag="ln1_sum_r")
                    q1_ps = psL1.tile([1, LQ], f32, tag="ln1_sum_s")
                    r1T = r1p.tile([128, DC, LQ], f32, tag="r1T")
                    with tc.tile_pool(name="psB", bufs=4, space="PSUM") as psB:
                        for f in range(DC):
                            xq_t = sm2.tile([128, 512], f32, tag="xq_t")
                            nc.sync.dma_start(xq_t, xq_v[:, f, :])
                            ps = psB.tile([128, 512], f32, tag="ao")
                            for cp in range(DC // 2):
                                nc.tensor.matmul(
                                    ps,
                                    wo_sb[:, f, cp * 256:(cp + 1) * 256]
                                    .rearrange("p (two f) -> p two f", two=2),
                                    ctxT[:, 2 * cp:2 * cp + 2, :],
                                    start=(cp == 0), stop=(cp == DC // 2 - 1),
                                    perf_mode=DR)
                            nc.vector.tensor_tensor(r1T[:, f, :], ps, xq_t, OP.add)
                            rb1 = sm2.tile([128, 512], bf16, tag="rb1")
                            nc.vector.tensor_copy(rb1, r1T[:, f, :])
                            sq1 = sm2.tile([128, 512], bf16, tag="sq1")
                            nc.vector.tensor_tensor(sq1, rb1, rb1, OP.mult)
                            nc.tensor.matmul(s1_ps, ones_bf, rb1,
                                             start=(f == 0), stop=(f == DC - 1))
                            nc.tensor.matmul(q1_ps, ones_bf, sq1,
                                             start=(f == 0), stop=(f == DC - 1))

                    # ---- LN1 stats + normalize (chunkwise) -> hT, hb ----
                    mu = sm.tile([1, LQ], f32, tag="ln_mu")
                    nc.scalar.activation(mu, s1_ps, AF.Copy, scale=1.0 / D)
                    msq = sm.tile([1, LQ], f32, tag="ln_msq")
                    nc.scalar.activation(msq, q1_ps, AF.Copy, scale=1.0 / D)
                    var = sm.tile([1, LQ], f32, tag="ln_var")
                    nc.vector.tensor_tensor(var, mu, mu, OP.mult)
                    nc.vector.tensor_tensor(var, msq, var, OP.subtract)
                    std = sm.tile([1, LQ], f32, tag="ln_std")
                    nc.scalar.activation(std, var, AF.Sqrt, bias=eps_t)
                    mrrow = sm.tile([1, 2 * LQ], f32, tag="ln_mrrow")
                    nc.vector.reciprocal_approx_fast(mrrow[:, LQ:2 * LQ], std)
                    nc.vector.tensor_copy(mrrow[:, 0:LQ], mu)
                    mr_ps = psM1.tile([128, 2 * LQ], f32, tag="ln_mrps")
                    for j in range(2):
                        nc.tensor.matmul(mr_ps[:, j * LQ:(j + 1) * LQ], ones_row,
                                         mrrow[:, j * LQ:(j + 1) * LQ],
                                         start=True, stop=True)
                    mu_bc, rstd_bc = mr_ps[:, 0:LQ], mr_ps[:, LQ:2 * LQ]
                    for c in range(DC):
                        cen = cen1p.tile([128, LQ], f32, tag="ln_cen")
                        nc.vector.tensor_tensor(cen, r1T[:, c, :], mu_bc, OP.subtract)
                        nc.vector.tensor_tensor(cen, cen, rstd_bc, OP.mult)
                        nc.scalar.activation(hT[:, c, :], cen, AF.Identity,
                                             scale=g1_sb[:, c:c + 1], bias=b1_sb[:, c:c + 1])
                        nc.vector.tensor_copy(hb[:, c, :], hT[:, c, :])
            # ---- Phase 3: FFN ----
            with tc.tile_pool(name="ffn", bufs=1) as ffn, \
                 tc.tile_pool(name="w1stream", bufs=4) as w1stream, \
                 tc.tile_pool(name="w2pool", bufs=1) as w2pool:
                g_sb = ffn.tile([128, FC, LQ], bf16, tag="g")
                r2T = ffn.tile([128, DC, LQ], f32, tag="r2T")
                w2_sb = w2pool.tile([128, FC, D], bf16, tag="w2_sb")

                with tc.tile_pool(name="psL2", bufs=1, space="PSUM") as psL2:
                  s2_ps = psL2.tile([1, LQ], f32, tag="ln2_sum_r")
                  q2_ps = psL2.tile([1, LQ], f32, tag="ln2_sum_s")
                  with tc.tile_pool(name="psZO", bufs=1, space="PSUM") as psZO:
                    for half in range(2):
                        o_ps = [psZO.tile([128, 512], f32, tag=f"o{f}", name=f"o_ps{f}")
                                for f in range(4)]
                        for i in range(FC):
                            if half == 0:
                                w1t = w1stream.tile([128, D], f8, tag="w1t")
                                nc.sync.dma_start(w1t, w1_d[i])
                                nc.sync.dma_start(w2_sb[:, i, :], w2_d[i])
                                zt = psZO.tile([128, 512], f32, tag=f"zt{i % 2}",
                                               name=f"zt{i % 2}")
                                for cp in range(DC // 2):
                                    nc.tensor.matmul(
                                        zt,
                                        w1t[:, cp * 256:(cp + 1) * 256]
                                        .rearrange("p (two f) -> p two f", two=2),
                                        hb[:, 2 * cp:2 * cp + 2, :],
                                        start=(cp == 0), stop=(cp == DC // 2 - 1),
                                        perf_mode=DR)
                                nc.scalar.activation(g_sb[:, i, :], zt, AF.Gelu,
                                                     bias=bb1_sb[:, i:i + 1])
                            for f in range(4):
                                nc.tensor.matmul(o_ps[f],
                                                 w2_sb[:, i, half * 512 + f * 128:half * 512 + (f + 1) * 128],
                                                 g_sb[:, i, :], start=(i == 0), stop=(i == FC - 1))
                        for f in range(4):
                            fo = half * 4 + f
                            t = sm2.tile([128, 512], f32, tag="obias")
                            nc.scalar.activation(t, o_ps[f], AF.Identity, bias=bb2_sb[:, fo:fo + 1])
                            nc.vector.tensor_tensor(r2T[:, fo, :], t, hT[:, fo, :], OP.add)
                            # LN2 prep folded in: bf16 copy + square + partial sums
                            rb2 = sm2.tile([128, 512], bf16, tag="rb2")
                            nc.vector.tensor_copy(rb2, r2T[:, fo, :])
                            sq2 = sm2.tile([128, 512], bf16, tag="sq2")
                            nc.vector.tensor_tensor(sq2, rb2, rb2, OP.mult)
                            nc.tensor.matmul(s2_ps, ones_bf, rb2,
                                             start=(fo == 0), stop=(fo == D // 128 - 1))
                            nc.tensor.matmul(q2_ps, ones_bf, sq2,
                                             start=(fo == 0), stop=(fo == D // 128 - 1))

                  # ---- LN2 stats + normalize -> out (chunked DMA) ----
                  with tc.tile_pool(name="ln2out", bufs=3) as ln2out, \
                       tc.tile_pool(name="psM2", bufs=1, space="PSUM") as psM2:
                      mu = sm.tile([1, LQ], f32, tag="ln_mu")
                      nc.scalar.activation(mu, s2_ps, AF.Copy, scale=1.0 / D)
                      msq = sm.tile([1, LQ], f32, tag="ln_msq")
                      nc.scalar.activation(msq, q2_ps, AF.Copy, scale=1.0 / D)
                      var = sm.tile([1, LQ], f32, tag="ln_var")
                      nc.vector.tensor_tensor(var, mu, mu, OP.mult)
                      nc.vector.tensor_tensor(var, msq, var, OP.subtract)
                      std = sm.tile([1, LQ], f32, tag="ln_std")
                      nc.scalar.activation(std, var, AF.Sqrt, bias=eps_t)
                      mrrow = sm.tile([1, 2 * LQ], f32, tag="ln_mrrow")
                      nc.vector.reciprocal_approx_fast(mrrow[:, LQ:2 * LQ], std)
                      nc.vector.tensor_copy(mrrow[:, 0:LQ], mu)
                      mr_ps = psM2.tile([128, 2 * LQ], f32, tag="ln_mrps")
                      for j in range(2):
                          nc.tensor.matmul(mr_ps[:, j * LQ:(j + 1) * LQ], ones_row,
                                           mrrow[:, j * LQ:(j + 1) * LQ],
                                           start=True, stop=True)
                      mu_bc, rstd_bc = mr_ps[:, 0:LQ], mr_ps[:, LQ:2 * LQ]
                      for c in range(DC):
                          cen = ln2out.tile([128, LQ], f32, tag="ln_cen")
                          nc.vector.tensor_tensor(cen, r2T[:, c, :], mu_bc, OP.subtract)
                          nc.vector.tensor_tensor(cen, cen, rstd_bc, OP.mult)
                          oc = ln2out.tile([128, LQ], f32, tag="ln_oc")
                          nc.scalar.activation(oc, cen, AF.Identity,
                                               scale=g2_sb[:, c:c + 1], bias=b2_sb[:, c:c + 1])
                          nc.sync.dma_start(out_v[:, c, :], oc)

    nc.compile()
    return nc


def _get_nc():
    if "nc" not in _cache:
        _cache["nc"] = _build_nc()
    return _cache["nc"]


def _host_prep(inputs):
    x = np.asarray(inputs["x"], np.float32)
    wq = np.asarray(inputs["wq"], np.float32)
    wk = np.asarray(inputs["wk"], np.float32)
    wo = np.asarray(inputs["wo"], np.float32)
    g1 = np.asarray(inputs["g1"], np.float32)
    b1 = np.asarray(inputs["b1"], np.float32)
    w1 = np.asarray(inputs["w1"], np.float32)
    bb1 = np.asarray(inputs["bb1"], np.float32)
    w2 = np.asarray(inputs["w2"], np.float32)
    bb2 = np.asarray(inputs["bb2"], np.float32)
    g2 = np.asarray(inputs["g2"], np.float32)
    b2 = np.asarray(inputs["b2"], np.float32)

    idx = np.arange(D)
    perm = (idx % HD) * NH + (idx // HD)  # f' = h*64+d  ->  old f = d*16+h

    def bf(a):
        return np.ascontiguousarray(a).astype(BF16NP)

    def f8(a):
        return np.ascontiguousarray(a).astype(F8NP)

    w1t = w1.reshape(DC, 128, FC, 128).transpose(2, 1, 0, 3).reshape(FC, 128, D)
    w2t = w2.reshape(FC, 128, D)
    def coblock(a):  # [K, F] -> [F-chunk, K-part, K-chunk-major cols]
        return a.reshape(DC, 128, DC, 128).transpose(2, 1, 0, 3).reshape(DC, 128, D)

    shared = {
        "wq": f8(coblock(wq[:, perm])),
        "wk": f8(coblock(wk[:, perm])),
        "wo": f8(coblock(wo[perm, :])),
        "w1": f8(w1t), "w2": bf(w2t),
        "ident": bf(np.eye(128, dtype=np.float32)),
        "bb1": bb1, "bb2": bb2, "g1": g1, "b1": b1, "g2": g2, "b2": b2,
    }
    in_maps = []
    for c in range(NCORES):
        b, q0 = c // (NCORES // B), (c % (NCORES // B)) * LQ
        xT = np.ascontiguousarray(x[b].T)
        m = dict(shared)
        m["xb"] = f8(np.ascontiguousarray(
            xT.reshape(D, 4, 512).transpose(1, 0, 2)))
        m["xqb"] = f8(xT[:, q0:q0 + LQ])
        m["xq"] = np.ascontiguousarray(xT[:, q0:q0 + LQ])
        in_maps.append(m)
    return in_maps


def kernel(**inputs):
    global LAST_RESULTS
    from concourse.bass_utils import run_bass_kernel_spmd

    nc = _get_nc()
    in_maps = _host_prep(inputs)
    res = run_bass_kernel_spmd(nc, in_maps, core_ids=list(range(NCORES)))
    LAST_RESULTS = res
    out = np.empty((B, L, D), np.float32)
    for c in range(NCORES):
        b, q0 = c // (NCORES // B), (c % (NCORES // B)) * LQ
        out[b, q0:q0 + LQ, :] = res.results[c]["out"].T
    return out


# revision 18
# speedup vs baseline: 1.3226x; 1.1430x over previous
"""Trainium2 Bass kernel for nn_EncoderLayer (B=2, L=2048, D=1024, 16 heads, FFN 4096).

Strategy: sequence-parallel over the 8 cores (core c owns batch c//4, query rows
(c%4)*512 .. +512).  Each core recomputes the full K projection for its batch,
which avoids all collectives; everything else is local.

v3: fp8 (e4m3) DoubleRow matmuls for the q/k projections, attn@K, wo and w1 —
2x PE throughput (256-wide contraction per 512-cycle instruction).  Scores stay
bf16 (column-bound, no fp8 gain) and w2 stays bf16 (fp8 would eat the error
budget).  Residual/LN paths stay fp32.  The kernel is restructured as a
per-head-pair pipeline: K-proj chunk co -> PE transposes -> scores/exp/attn@K
for heads 2co,2co+1, so the ACT engine (exp is the attention bottleneck) fills
from ~6us in and PE projection work hides in ACT-bound slack.  The softmax
1/sqrt(HD) scale is applied in the exp activation (scale=0.125) instead of
pre-scaling wq, which would push fp8 weights into subnormals.
Softmax denominators: per-pair reciprocal + DRAM partition-broadcast, hidden
under the next pair's compute.  LN1 normalize is emitted interleaved with the
FFN w1 stream; LN2 tail is pipelined into chunked output DMA.
"""

import sys
sys.setrecursionlimit(200000)
import numpy as np
import ml_dtypes

B, L, D, NH, HD, FF = 2, 2048, 1024, 16, 64, 4096
LQ = 512  # query rows per core
NCORES = 8
EPS = 1e-5
DC = D // 128  # 8 feature chunks
MC = L // 128  # 16 key chunks
FC = FF // 128  # 32 ffn chunks
BF16NP = ml_dtypes.bfloat16
F8NP = ml_dtypes.float8_e4m3

_cache = {}
LAST_RESULTS = None


def _build_nc():
    import concourse.bass as bass
    import concourse.tile as tile
    from concourse import bacc, mybir
    from contextlib import ExitStack

    f32 = mybir.dt.float32
    bf16 = mybir.dt.bfloat16
    f8 = mybir.dt.float8e4
    AF = mybir.ActivationFunctionType
    OP = mybir.AluOpType
    DR = mybir.MatmulPerfMode.DoubleRow

    nc = bacc.Bacc("TRN2", debug=False, target_bir_lowering=False)

    # ---- DRAM I/O ----
    xb_d = nc.dram_tensor("xb", [4, D, 512], f8, kind="ExternalInput").ap()
    xqb_d = nc.dram_tensor("xqb", [D, LQ], f8, kind="ExternalInput").ap()
    xq_d = nc.dram_tensor("xq", [D, LQ], f32, kind="ExternalInput").ap()
    wq_d = nc.dram_tensor("wq", [DC, 128, D], f8, kind="ExternalInput").ap()
    wk_d = nc.dram_tensor("wk", [DC, 128, D], f8, kind="ExternalInput").ap()
    wo_d = nc.dram_tensor("wo", [DC, 128, D], f8, kind="ExternalInput").ap()
    w1_d = nc.dram_tensor("w1", [FC, 128, D], f8, kind="ExternalInput").ap()
    w2_d = nc.dram_tensor("w2", [FC // 2, 128, 2 * D], f8, kind="ExternalInput").ap()
    ident_d = nc.dram_tensor("ident", [128, 128], bf16, kind="ExternalInput").ap()
    bb1_d = nc.dram_tensor("bb1", [FF], f32, kind="ExternalInput").ap()
    bb2_d = nc.dram_tensor("bb2", [D], f32, kind="ExternalInput").ap()
    g1_d = nc.dram_tensor("g1", [D], f32, kind="ExternalInput").ap()
    b1_d = nc.dram_tensor("b1", [D], f32, kind="ExternalInput").ap()
    g2_d = nc.dram_tensor("g2", [D], f32, kind="ExternalInput").ap()
    b2_d = nc.dram_tensor("b2", [D], f32, kind="ExternalInput").ap()
    out_d = nc.dram_tensor("out", [D, LQ], f32, kind="ExternalOutput").ap()

    xqb_v = xqb_d.rearrange("(c p) l -> p c l", p=128)
    xq_v = xq_d.rearrange("(c p) l -> p c l", p=128)
    bb1_v = bb1_d.rearrange("(c p) -> p c", p=128)
    bb2_v = bb2_d.rearrange("(c p) -> p c", p=128)
    g1_v = g1_d.rearrange("(c p) -> p c", p=128)
    b1_v = b1_d.rearrange("(c p) -> p c", p=128)
    g2_v = g2_d.rearrange("(c p) -> p c", p=128)
    b2_v = b2_d.rearrange("(c p) -> p c", p=128)
    out_v = out_d.rearrange("(c p) l -> p c l", p=128)

    with tile.TileContext(nc, pool_alloc_mode="queue") as tc, ExitStack() as top:
        consts = top.enter_context(tc.tile_pool(name="consts", bufs=1))
        dramsc = top.enter_context(tc.tile_pool(name="dramsc", bufs=2, space="DRAM"))

        sm = top.enter_context(tc.tile_pool(name="smalls", bufs=1))
        sm2 = top.enter_context(tc.tile_pool(name="smalls2", bufs=2))

        with tc.tile_pool(name="mid", bufs=1) as mid:
            hT = mid.tile([128, DC, LQ], f32, tag="hT")
            hb = mid.tile([128, DC, LQ], f8, tag="hb")

            with tc.tile_pool(name="kq", bufs=1) as kq:
                kT = kq.tile([128, DC, L], bf16, tag="kT")
                # kaug[p, mj, h, i*128 + j]: m-chunk pair mj, head h, k-tile i
                # (m = 2*mj+i), col j in [0:64] = head dims, 64 = ones, rest pad
                kaug = kq.tile([128, MC // 2, NH, 256], f8, tag="kaug")
                qT = kq.tile([128, DC, LQ], bf16, tag="qT")
                ctxT = kq.tile([128, DC, LQ], f8, tag="ctxT")

                # ---- Phase 1+2: interleaved projections + attention ----
                with tc.tile_pool(name="p1", bufs=1) as p1, \
                     tc.tile_pool(name="p1w", bufs=1) as p1w, \
                     tc.tile_pool(name="epool", bufs=2) as epool, \
                     tc.tile_pool(name="cpool", bufs=2) as cpool, \
                     tc.tile_pool(name="wop", bufs=1) as wop, \
                     tc.tile_pool(name="psK", bufs=2, space="PSUM") as psK, \
                     tc.tile_pool(name="psT", bufs=1, space="PSUM") as psT, \
                     tc.tile_pool(name="psS", bufs=2, space="PSUM") as psS, \
                     tc.tile_pool(name="psU", bufs=1, space="PSUM") as psU:
    # chunk-0 inputs first so the tensor engine starts early; the rest of the
                    # weight chunks stream behind xb so head-pair co's inputs
                    # land just in time
                    xqb = p1.tile([128, DC, LQ], f8, tag="xqb")
                    nc.sync.dma_start(xqb, xqb_v)
                    wq_sb = p1w.tile([128, DC, D], f8, tag="wproj")
                    wk_sb = p1w.tile([128, DC, D], f8, tag="wproj_k")
                    xb = p1.tile([128, 4, DC, 512], f8, tag="xb")
                    nc.sync.dma_start(wq_sb[:, 0, :], wq_d[0])
                    nc.sync.dma_start(wk_sb[:, 0, :], wk_d[0])
                    for mt in range(4):
                        nc.sync.dma_start(
                            xb[:, mt, :, :],
                            xb_d[mt].rearrange("(c p) m -> p c m", p=128))
                    ident = consts.tile([128, 128], bf16, tag="ident")
                    nc.sync.dma_start(ident, ident_d)
                    for co in range(1, DC):
                        nc.sync.dma_start(wq_sb[:, co, :], wq_d[co])
                        nc.sync.dma_start(wk_sb[:, co, :], wk_d[co])

                    # constants (small DMAs, off the critical path)
                    ones_bf = consts.tile([128, 1], bf16, tag="ones")
                    nc.vector.memset(ones_bf, 1.0)
                    ones_row = consts.tile([1, 128], f32, tag="ones_row")
                    nc.vector.memset(ones_row, 1.0)
                    eps_t = consts.tile([1, 1], f32, tag="eps")
                    nc.vector.memset(eps_t, EPS)
                    bb1_sb = consts.tile([128, FC], f32, tag="bb1")
                    nc.sync.dma_start(bb1_sb, bb1_v)
                    bb2_sb = consts.tile([128, DC], f32, tag="bb2")
                    nc.sync.dma_start(bb2_sb, bb2_v)
                    g1_sb = consts.tile([128, DC], f32, tag="g1")
                    nc.sync.dma_start(g1_sb, g1_v)
                    b1_sb = consts.tile([128, DC], f32, tag="b1")
                    nc.sync.dma_start(b1_sb, b1_v)
                    g2_sb = consts.tile([128, DC], f32, tag="g2")
                    nc.sync.dma_start(g2_sb, g2_v)
                    b2_sb = consts.tile([128, DC], f32, tag="b2")
                    nc.sync.dma_start(b2_sb, b2_v)
                    kaug_b = kaug.rearrange("p mj h (two f) -> p mj (h two) f",
                                            two=2)
                    # ones column = 8.0: wk is host-scaled by 8, so kT holds
                    # 8*k; den row becomes 8*sum(e), cancelling the 8 in ctx
                    nc.vector.memset(kaug_b[:, :, :, 64:65], 8.0)
                    nc.vector.memset(kaug_b[:, :, :, 65:128], 0.0)

                    wo_sb = wop.tile([128, DC, D], f8, tag="wo_sb")
                    scd = dramsc.tile([NH, LQ], bf16, tag="rec_sc")

                    for co in range(DC):
                        if co == 2:
                            # prefetch wo once the input stream has drained
                            for cw in range(DC):
                                nc.sync.dma_start(wo_sb[:, cw, :], wo_d[cw])
                        # ---- q chunk co ----
                        psq = psK.tile([128, 512], f32, tag="psk")
                        for cp in range(DC // 2):
                            nc.tensor.matmul(
                                psq,
                                wq_sb[:, co, cp * 256:(cp + 1) * 256]
                                .rearrange("p (two f) -> p two f", two=2),
                                xqb[:, 2 * cp:2 * cp + 2, :],
                                start=(cp == 0), stop=(cp == DC // 2 - 1),
                                perf_mode=DR)
                        nc.vector.tensor_copy(qT[:, co, :], psq)

                        # ---- k chunk co over full L ----
                        for mt in range(4):
                            ps = psK.tile([128, 512], f32, tag="psk")
                            for cp in range(DC // 2):
                                nc.tensor.matmul(
                                    ps,
                                    wk_sb[:, co, cp * 256:(cp + 1) * 256]
                                    .rearrange("p (two f) -> p two f", two=2),
                                    xb[:, mt, 2 * cp:2 * cp + 2, :],
                                    start=(cp == 0), stop=(cp == DC // 2 - 1),
                                    perf_mode=DR)
                            nc.vector.tensor_copy(
                                kT[:, co, mt * 512:(mt + 1) * 512], ps)

                        # ---- transposes -> kaug for heads 2co, 2co+1 ----
                        for g in range(2):
                            pt = psT.tile([128, 1024], bf16, tag="pt")
                            for j in range(8):
                                mi = g * 8 + j
                                nc.tensor.transpose(
                                    pt[:, j * 128:(j + 1) * 128],
                                    kT[:, co, mi * 128:(mi + 1) * 128], ident)
                            ptv = pt.rearrange("p (m he) -> p m he", he=128)
                            for s in range(2):
                                for i in range(2):
                                    # m-chunks g*8+i, g*8+i+2, ... (parity i)
                                    nc.vector.tensor_copy(
                                        kaug[:, g * 4:(g + 1) * 4, 2 * co + s,
                                             i * 128:i * 128 + 64],
                                        ptv[:, i::2, s * 64:(s + 1) * 64])

                        # ---- heads 2co, 2co+1 ----
                        cT = cpool.tile([128, LQ], bf16, tag="cT")
                        den_bc = cpool.tile([128, LQ], bf16, tag="den_bc")
                        for s in range(2):
                            h = 2 * co + s
                            poff = 64 * s
                            e = epool.tile([128, MC, LQ], f8, tag="E")
                            for mt in range(MC // 2):
                                st = psS.tile([128, 1024], f32, tag="st")
                                for j in range(2):
                                    mi = mt * 2 + j
                                    nc.tensor.matmul(
                                        st[:, j * 512:(j + 1) * 512],
                                        kT[poff:poff + 64, co,
                                           mi * 128:(mi + 1) * 128],
                                        qT[poff:poff + 64, co, :],
                                        start=True, stop=True)
                                # wq,wk host-scaled by 8 => scores are 64x;
                                # fold 1/sqrt(HD)/64 = 1/512 into the exp
                                nc.scalar.activation(
                                    e[:, mt * 2:(mt + 1) * 2, :]
                                    .rearrange("p a b -> p (a b)"),
                                    st, AF.Exp, scale=1.0 / 512.0)
                            u = psU.tile([128, 512], f32, tag="u")
                            for mj in range(MC // 2):
                                nc.tensor.matmul(
                                    u, kaug[:, mj, h, :]
                                    .rearrange("p (two f) -> p two f", two=2),
                                    e[:, 2 * mj:2 * mj + 2, :],
                                    start=(mj == 0), stop=(mj == MC // 2 - 1),
                                    perf_mode=DR)
                            nc.vector.tensor_copy(cT[poff:poff + 64, :],
                                                  u[0:64, :])
                            drow = sm2.tile([1, LQ], f32, tag="drow")
                            nc.vector.tensor_copy(drow, u[64:65, :])
                            rec32 = sm2.tile([1, LQ], f32, tag="rec32")
                            nc.vector.reciprocal_approx_fast(rec32, drow)
                            rec16 = sm2.tile([1, LQ], bf16, tag="rec16")
                            nc.vector.tensor_copy(rec16, rec32)
                            nc.sync.dma_start(scd[h:h + 1, :], rec16)
                            nc.sync.dma_start(
                                den_bc[poff:poff + 64, :],
                                scd[h:h + 1, :].partition_broadcast(64))
                        nc.vector.tensor_tensor(ctxT[:, co, :], cT, den_bc,
                                                OP.mult)

                # ---- attn_out + residual -> r1T, with LN1 prep folded in ----
                with tc.tile_pool(name="r1p", bufs=1) as r1p, \
                     tc.tile_pool(name="psL1", bufs=1, space="PSUM") as psL1, \
                     tc.tile_pool(name="cen1p", bufs=2) as cen1p, \
                     tc.tile_pool(name="psM1", bufs=1, space="PSUM") as psM1:
                    s1_ps = psL1.tile([1, LQ], f32, tag="ln1_sum_r")
                    q1_ps = psL1.tile([1, LQ], f32, tag="ln1_sum_s")
                    r1T = r1p.tile([128, DC, LQ], f32, tag="r1T")
                    with tc.tile_pool(name="psB", bufs=4, space="PSUM") as psB:
                        for f in range(DC):
                            xq_t = sm2.tile([128, 512], f32, tag="xq_t")
                            nc.sync.dma_start(xq_t, xq_v[:, f, :])
                            ps = psB.tile([128, 512], f32, tag="ao")
                            for cp in range(DC // 2):
                                nc.tensor.matmul(
                                    ps,
                                    wo_sb[:, f, cp * 256:(cp + 1) * 256]
                                    .rearrange("p (two f) -> p two f", two=2),
                                    ctxT[:, 2 * cp:2 * cp + 2, :],
                                    start=(cp == 0), stop=(cp == DC // 2 - 1),
                                    perf_mode=DR)
                            nc.vector.tensor_tensor(r1T[:, f, :], ps, xq_t, OP.add)
                            rb1 = sm2.tile([128, 512], bf16, tag="rb1")
                            nc.vector.tensor_copy(rb1, r1T[:, f, :])
                            sq1 = sm2.tile([128, 512], bf16, tag="sq1")
                            nc.vector.tensor_tensor(sq1, rb1, rb1, OP.mult)
                            nc.tensor.matmul(s1_ps, ones_bf, rb1,
                                             start=(f == 0), stop=(f == DC - 1))
                            nc.tensor.matmul(q1_ps, ones_bf, sq1,
                                             start=(f == 0), stop=(f == DC - 1))

                    # ---- LN1 stats + normalize (chunkwise) -> hT, hb ----
                    mu = sm.tile([1, LQ], f32, tag="ln_mu")
                    nc.scalar.activation(mu, s1_ps, AF.Copy, scale=1.0 / D)
                    msq = sm.tile([1, LQ], f32, tag="ln_msq")
                    nc.scalar.activation(msq, q1_ps, AF.Copy, scale=1.0 / D)
                    var = sm.tile([1, LQ], f32, tag="ln_var")
                    nc.vector.tensor_tensor(var, mu, mu, OP.mult)
                    nc.vector.tensor_tensor(var, msq, var, OP.subtract)
                    std = sm.tile([1, LQ], f32, tag="ln_std")
                    nc.scalar.activation(std, var, AF.Sqrt, bias=eps_t)
                    mrrow = sm.tile([1, 2 * LQ], f32, tag="ln_mrrow")
                    nc.vector.reciprocal_approx_fast(mrrow[:, LQ:2 * LQ], std)
                    nc.vector.tensor_copy(mrrow[:, 0:LQ], mu)
                    mr_ps = psM1.tile([128, 2 * LQ], f32, tag="ln_mrps")
                    for j in range(2):
                        nc.tensor.matmul(mr_ps[:, j * LQ:(j + 1) * LQ], ones_row,
                                         mrrow[:, j * LQ:(j + 1) * LQ],
                                         start=True, stop=True)
                    mu_bc, rstd_bc = mr_ps[:, 0:LQ], mr_ps[:, LQ:2 * LQ]
                    for c in range(DC):
                        cen = cen1p.tile([128, LQ], f32, tag="ln_cen")
                        nc.vector.tensor_tensor(cen, r1T[:, c, :], mu_bc, OP.subtract)
                        nc.vector.tensor_tensor(cen, cen, rstd_bc, OP.mult)
                        nc.scalar.activation(hb[:, c, :], cen, AF.Identity,
                                             scale=g1_sb[:, c:c + 1], bias=b1_sb[:, c:c + 1])
                        nc.scalar.activation(hT[:, c, :], cen, AF.Identity,
                                             scale=g1_sb[:, c:c + 1], bias=b1_sb[:, c:c + 1])
            # ---- Phase 3: FFN ----
            with tc.tile_pool(name="ffn", bufs=1) as ffn, \
                 tc.tile_pool(name="w1stream", bufs=4) as w1stream, \
                 tc.tile_pool(name="w2pool", bufs=1) as w2pool:
                g_sb = ffn.tile([128, FC, LQ], f8, tag="g")
                r2T = ffn.tile([128, DC, LQ], f32, tag="r2T")
                w2_sb = w2pool.tile([128, FC // 2, 2 * D], f8, tag="w2_sb")

                with tc.tile_pool(name="psL2", bufs=1, space="PSUM") as psL2:
                  s2_ps = psL2.tile([1, LQ], f32, tag="ln2_sum_r")
                  q2_ps = psL2.tile([1, LQ], f32, tag="ln2_sum_s")
                  with tc.tile_pool(name="psZO", bufs=1, space="PSUM") as psZO:
                    for half in range(2):
                        o_ps = [psZO.tile([128, 512], f32, tag=f"o{f}", name=f"o_ps{f}")
                                for f in range(4)]
                        for j in range(FC // 2):
                            if half == 0:
                                for i in (2 * j, 2 * j + 1):
                                    w1t = w1stream.tile([128, D], f8, tag="w1t")
                                    nc.sync.dma_start(w1t, w1_d[i])
                                    zt = psZO.tile([128, 512], f32, tag=f"zt{i % 2}",
                                                   name=f"zt{i % 2}")
                                    for cp in range(DC // 2):
                                        nc.tensor.matmul(
                                            zt,
                                            w1t[:, cp * 256:(cp + 1) * 256]
                                            .rearrange("p (two f) -> p two f", two=2),
                                            hb[:, 2 * cp:2 * cp + 2, :],
                                            start=(cp == 0), stop=(cp == DC // 2 - 1),
                                            perf_mode=DR)
                                    # w1 host-scaled by 8: z = zt/8 + bb1
                                    nc.scalar.activation(g_sb[:, i, :], zt, AF.Gelu,
                                                         scale=0.125,
                                                         bias=bb1_sb[:, i:i + 1])
                                nc.sync.dma_start(w2_sb[:, j, :], w2_d[j])
                            for f in range(4):
                                fo = half * 4 + f
                                nc.tensor.matmul(
                                    o_ps[f],
                                    w2_sb[:, j, fo * 256:(fo + 1) * 256]
                                    .rearrange("p (two f) -> p two f", two=2),
                                    g_sb[:, 2 * j:2 * j + 2, :],
                                    start=(j == 0), stop=(j == FC // 2 - 1),
                                    perf_mode=DR)
                        for f in range(4):
                            fo = half * 4 + f
                            t = sm2.tile([128, 512], f32, tag="obias")
                            # w2 host-scaled by 16: o = o_ps/16 + bb2
                            nc.scalar.activation(t, o_ps[f], AF.Identity,
                                                 scale=0.0625,
                                                 bias=bb2_sb[:, fo:fo + 1])
                            nc.vector.tensor_tensor(r2T[:, fo, :], t, hT[:, fo, :], OP.add)
                            # LN2 prep folded in: bf16 copy + square + partial sums
                            rb2 = sm2.tile([128, 512], bf16, tag="rb2")
                            nc.vector.tensor_copy(rb2, r2T[:, fo, :])
                            sq2 = sm2.tile([128, 512], bf16, tag="sq2")
                            nc.vector.tensor_tensor(sq2, rb2, rb2, OP.mult)
                            nc.tensor.matmul(s2_ps, ones_bf, rb2,
                                             start=(fo == 0), stop=(fo == D // 128 - 1))
                            nc.tensor.matmul(q2_ps, ones_bf, sq2,
                                             start=(fo == 0), stop=(fo == D // 128 - 1))

                  # ---- LN2 stats + normalize -> out (chunked DMA) ----
                  with tc.tile_pool(name="ln2out", bufs=3) as ln2out, \
                       tc.tile_pool(name="psM2", bufs=1, space="PSUM") as psM2:
                      mu = sm.tile([1, LQ], f32, tag="ln_mu")
                      nc.scalar.activation(mu, s2_ps, AF.Copy, scale=1.0 / D)
                      msq = sm.tile([1, LQ], f32, tag="ln_msq")
                      nc.scalar.activation(msq, q2_ps, AF.Copy, scale=1.0 / D)
                      var = sm.tile([1, LQ], f32, tag="ln_var")
                      nc.vector.tensor_tensor(var, mu, mu, OP.mult)
                      nc.vector.tensor_tensor(var, msq, var, OP.subtract)
                      std = sm.tile([1, LQ], f32, tag="ln_std")
                      nc.scalar.activation(std, var, AF.Sqrt, bias=eps_t)
                      mrrow = sm.tile([1, 2 * LQ], f32, tag="ln_mrrow")
                      nc.vector.reciprocal_approx_fast(mrrow[:, LQ:2 * LQ], std)
                      nc.vector.tensor_copy(mrrow[:, 0:LQ], mu)
                      mr_ps = psM2.tile([128, 2 * LQ], f32, tag="ln_mrps")
                      for j in range(2):
                          nc.tensor.matmul(mr_ps[:, j * LQ:(j + 1) * LQ], ones_row,
                                           mrrow[:, j * LQ:(j + 1) * LQ],
                                           start=True, stop=True)
                      mu_bc, rstd_bc = mr_ps[:, 0:LQ], mr_ps[:, LQ:2 * LQ]
                      for c in range(DC):
                          cen = ln2out.tile([128, LQ], f32, tag="ln_cen")
                          nc.vector.tensor_tensor(cen, r2T[:, c, :], mu_bc, OP.subtract)
                          nc.vector.tensor_tensor(cen, cen, rstd_bc, OP.mult)
                          oc = ln2out.tile([128, LQ], f32, tag="ln_oc")
                          nc.scalar.activation(oc, cen, AF.Identity,
                                               scale=g2_sb[:, c:c + 1], bias=b2_sb[:, c:c + 1])
                          nc.sync.dma_start(out_v[:, c, :], oc)

    nc.compile()
    return nc


def _get_nc():
    if "nc" not in _cache:
        _cache["nc"] = _build_nc()
    return _cache["nc"]


def _host_prep(inputs):
    x = np.asarray(inputs["x"], np.float32)
    wq = np.asarray(inputs["wq"], np.float32)
    wk = np.asarray(inputs["wk"], np.float32)
    wo = np.asarray(inputs["wo"], np.float32)
    g1 = np.asarray(inputs["g1"], np.float32)
    b1 = np.asarray(inputs["b1"], np.float32)
    w1 = np.asarray(inputs["w1"], np.float32)
    bb1 = np.asarray(inputs["bb1"], np.float32)
    w2 = np.asarray(inputs["w2"], np.float32)
    bb2 = np.asarray(inputs["bb2"], np.float32)
    g2 = np.asarray(inputs["g2"], np.float32)
    b2 = np.asarray(inputs["b2"], np.float32)

    idx = np.arange(D)
    perm = (idx % HD) * NH + (idx // HD)  # f' = h*64+d  ->  old f = d*16+h

    def bf(a):
        return np.ascontiguousarray(a).astype(BF16NP)

    def f8(a):
        return np.ascontiguousarray(a).astype(F8NP)

    # w1 x8 / w2 x16 / wq,wk x8: power-of-2 pre-scales keep fp8 weights out of
    # the subnormal range; compensated on-device (gelu/obias/exp scale args,
    # kaug ones column = 8)
    w1t = (8.0 * w1).reshape(DC, 128, FC, 128).transpose(2, 1, 0, 3).reshape(FC, 128, D)
    # w2p[j, p, fc*256 + i*128 + c] = 16*w2[(2j+i)*128+p, fc*128+c]
    w2p = (16.0 * w2).reshape(FC // 2, 2, 128, DC, 128).transpose(0, 2, 3, 1, 4) \
        .reshape(FC // 2, 128, 2 * D)
    def coblock(a):  # [K, F] -> [F-chunk, K-part, K-chunk-major cols]
        return a.reshape(DC, 128, DC, 128).transpose(2, 1, 0, 3).reshape(DC, 128, D)

    shared = {
        "wq": f8(coblock(8.0 * wq[:, perm])),
        "wk": f8(coblock(8.0 * wk[:, perm])),
        "wo": f8(coblock(wo[perm, :])),
        "w1": f8(w1t), "w2": f8(w2p),
        "ident": bf(np.eye(128, dtype=np.float32)),
        "bb1": bb1, "bb2": bb2, "g1": g1, "b1": b1, "g2": g2, "b2": b2,
    }
    in_maps = []
    for c in range(NCORES):
        b, q0 = c // (NCORES // B), (c % (NCORES // B)) * LQ
        xT = np.ascontiguousarray(x[b].T)
        m = dict(shared)
        m["xb"] = f8(np.ascontiguousarray(
            xT.reshape(D, 4, 512).transpose(1, 0, 2)))
        m["xqb"] = f8(xT[:, q0:q0 + LQ])
        m["xq"] = np.ascontiguousarray(xT[:, q0:q0 + LQ])
        in_maps.append(m)
    return in_maps


def kernel(**inputs):
    global LAST_RESULTS
    from concourse.bass_utils import run_bass_kernel_spmd

    nc = _get_nc()
    in_maps = _host_prep(inputs)
    res = run_bass_kernel_spmd(nc, in_maps, core_ids=list(range(NCORES)))
    LAST_RESULTS = res
    out = np.empty((B, L, D), np.float32)
    for c in range(NCORES):
        b, q0 = c // (NCORES // B), (c % (NCORES // B)) * LQ
        out[b, q0:q0 + LQ, :] = res.results[c]["out"].T
    return out
